# revision 3
# baseline (speedup 1.0000x reference)
"""Windowed sparse attention kernel for TRN2 (8 NeuronCores).

Problem: b=1, h=16, n=16384, d=32, window w=128, nw=128 windows.
Each window of 128 queries attends to [4 memory slots | prev window | cur window]
with additive bias, tanh softcap (50), softmax.

Sharding: sequence-parallel over windows. Core c handles windows
[c*16, (c+1)*16) for all 16 heads, with a one-window k/v halo.

Device layout ("layout B", keys on partitions):
  simT[key, q] = sum_d k[key,d] * q[q,d]   (PE, K=d=32, 4 heads packed in
                                            128 partitions via row groups)
  DVE: simS = simT(PSUM) + biasT(SBUF)     (fused evacuation + bias add)
  ACT: t = tanh(simS / 50); p = exp(50 t)  (wide 4096-col instructions)
  PE:  outT~[0:33, q] += v~[key, 0:33]^T @ p[key, q]  per key-chunk,
       where v~ has a ones column -> row 32 of outT~ is the softmax
       denominator Z (window part).
Host combines the 4-slot memory attention (tiny: 1.5% of keys) and
normalizes: out = (out_win_u + out_mem_u) / (Z_win + Z_mem).
"""

import numpy as np

B, H, N, D = 1, 16, 16384, 32
W = 128                 # window size
NW = N // W             # 128 windows
NCORES = 8
WPC = NW // NCORES      # 16 windows per core
NSLOT = WPC + 1         # 17 k/v slots (halo)
SOFTCLAMP = 50.0
SCALE = D ** -0.5
MASK_PEN = -30000.0

_COMPILED = None


def _build_bass():
    import concourse.bacc as bacc
    import concourse.tile as tile
    from concourse import mybir
    from contextlib import ExitStack

    f32 = mybir.dt.float32
    nc = bacc.Bacc()

    qT = nc.declare_dram_parameter("qT", [4, 128, WPC * W], f32, isOutput=False)
    kT = nc.declare_dram_parameter("kT", [4, 128, NSLOT * W], f32, isOutput=False)
    vv = nc.declare_dram_parameter("vv", [H, 128, NSLOT * 33], f32, isOutput=False)
    bT = nc.declare_dram_parameter("bT", [128, WPC * 2 * W], f32, isOutput=False)
    o = nc.declare_dram_parameter("o", [H, 33, WPC * W], f32, isOutput=True)

    with ExitStack() as ctx:
        tc = ctx.enter_context(tile.TileContext(nc))
        singles = ctx.enter_context(tc.tile_pool(name="singles", bufs=1))
        qk_pool = ctx.enter_context(tc.tile_pool(name="qk", bufs=2))
        v_pool = ctx.enter_context(tc.tile_pool(name="v", bufs=2))
        wide = ctx.enter_context(tc.tile_pool(name="wide", bufs=2))
        ow_pool = ctx.enter_context(tc.tile_pool(name="ow", bufs=2))
        sim_ps = ctx.enter_context(tc.tile_pool(name="simps", bufs=4, space="PSUM"))
        out_ps = ctx.enter_context(tc.tile_pool(name="outps", bufs=4, space="PSUM"))

        biasS = singles.tile([128, WPC * 2 * W], f32)
        nc.sync.dma_start(out=biasS[:, :], in_=bT[:, :])

        for g in range(4):
            Qg = qk_pool.tile([128, WPC * W], f32, tag="qg")
            nc.sync.dma_start(out=Qg[:, :], in_=qT[g])
            Kg = qk_pool.tile([128, NSLOT * W], f32, tag="kg")
            nc.sync.dma_start(out=Kg[:, :], in_=kT[g])
            for i in range(4):
                h = 4 * g + i
                p0 = 32 * i
                Vh = v_pool.tile([128, NSLOT * 33], f32)
                nc.sync.dma_start(out=Vh[:, :], in_=vv[h])

                simS = wide.tile([128, WPC * 2 * W], f32, tag="simS")
                # mm1 + bias add, one PSUM bank per two tasks (512 f32)
                for t2 in range(WPC // 2):
                    simP = sim_ps.tile([128, 512], f32)
                    for u in range(2):
                        w = 2 * t2 + u
                        for chunk in range(2):       # 0=prev(slot w) 1=cur(slot w+1)
                            slot = w + chunk
                            nc.tensor.matmul(
                                simP[:, u * 256 + chunk * 128:u * 256 + chunk * 128 + 128],
                                lhsT=Kg[p0:p0 + 32, slot * W:(slot + 1) * W],
                                rhs=Qg[p0:p0 + 32, w * W:(w + 1) * W],
                                start=True, stop=True,
                                tile_position=(p0, 0),
                            )
                    nc.vector.tensor_add(
                        simS[:, t2 * 512:(t2 + 1) * 512],
                        simP[:, :],
                        biasS[:, t2 * 512:(t2 + 1) * 512],
                    )
                # softcap + exp, wide
                tS = wide.tile([128, WPC * 2 * W], f32, tag="tS")
                nc.scalar.activation(tS[:, :], simS[:, :],
                                     mybir.ActivationFunctionType.Tanh,
                                     scale=1.0 / SOFTCLAMP)
                pS = wide.tile([128, WPC * 2 * W], f32, tag="pS")
                nc.scalar.activation(pS[:, :], tS[:, :],
                                     mybir.ActivationFunctionType.Exp,
                                     scale=SOFTCLAMP)
                # mm2: out~ (33, q) accumulated per 4 tasks into one bank
                outW = ow_pool.tile([33, WPC * W], f32)
                for t4 in range(WPC // 4):
                    otP = out_ps.tile([33, 512], f32)
                    for u in range(4):
                        w = 4 * t4 + u
                        for chunk in range(2):
                            slot = w + chunk
                            nc.tensor.matmul(
                                otP[:, u * 128:(u + 1) * 128],
                                lhsT=Vh[:, slot * 33:(slot + 1) * 33],
                                rhs=pS[:, w * 256 + chunk * 128:w * 256 + chunk * 128 + 128],
                                start=(chunk == 0), stop=(chunk == 1),
                            )
                    nc.vector.tensor_copy(outW[:, t4 * 512:(t4 + 1) * 512], otP[:, :])
                nc.sync.dma_start(out=o[h], in_=outW[:, :])
    nc.compile()
    return nc


def _get_compiled():
    global _COMPILED
    if _COMPILED is None:
        _COMPILED = _build_bass()
    return _COMPILED


def _prep_core(c, qs, ks, vs, ab, mvec):
    """Build per-core input arrays. qs,ks,vs: (H, N, D) (qs pre-scaled)."""
    w0 = c * WPC
    qw = qs.reshape(H, NW, W, D)[:, w0:w0 + WPC]          # (H,16,128,32)
    qTc = np.ascontiguousarray(
        qw.reshape(4, 4, WPC, W, D).transpose(0, 1, 4, 2, 3).reshape(4, 128, WPC * W))

    kw = ks.reshape(H, NW, W, D)
    vw = vs.reshape(H, NW, W, D)
    khalo = np.zeros((H, NSLOT, W, D), np.float32)
    vhalo = np.zeros((H, NSLOT, W, D), np.float32)
    lo = w0 - 1
    src_lo = max(lo, 0)
    dst_lo = src_lo - lo
    khalo[:, dst_lo:] = kw[:, src_lo:w0 + WPC]
    vhalo[:, dst_lo:] = vw[:, src_lo:w0 + WPC]
    kTc = np.ascontiguousarray(
        khalo.reshape(4, 4, NSLOT, W, D).transpose(0, 1, 4, 2, 3).reshape(4, 128, NSLOT * W))
    vvc = np.concatenate([vhalo, np.ones((H, NSLOT, W, 1), np.float32)], axis=3)
    vvc = np.ascontiguousarray(
        vvc.transpose(0, 2, 1, 3).reshape(H, 128, NSLOT * 33))

    bsl = ab[w0:w0 + WPC].reshape(WPC, W, 2, W)            # (w, q, chunk, key)
    bTc = np.ascontiguousarray(bsl.transpose(3, 0, 2, 1))  # (key, w, chunk, q)
    # fold key-position mask (incl. structural masking of window -1)
    kmask = np.ones((WPC, 2, W), bool)
    for w in range(WPC):
        for chunk in range(2):
            gw = w0 + w - 1 + chunk
            if gw < 0:
                kmask[w, chunk] = False
            else:
                kmask[w, chunk] = mvec[gw * W:(gw + 1) * W]
    pen = np.where(kmask, np.float32(0), np.float32(MASK_PEN))  # (w,chunk,key)
    bTc = bTc + pen.transpose(2, 0, 1)[:, :, :, None]
    bTc = np.ascontiguousarray(bTc.reshape(128, WPC * 2 * W), np.float32)
    return {"qT": qTc, "kT": kTc, "vv": vvc, "bT": bTc}


def _run_device(in_maps, trace=False):
    from concourse.bass_utils import run_bass_kernel_spmd
    nc = _get_compiled()
    res = run_bass_kernel_spmd(nc, in_maps, list(range(NCORES)), trace=trace)
    return res


def kernel(q, k, v, mask, attn_bias, memory_kv, _trace=False, _ret_res=False):
    q = np.asarray(q, np.float32)
    k = np.asarray(k, np.float32)
    v = np.asarray(v, np.float32)
    mask = np.asarray(mask)
    attn_bias = np.asarray(attn_bias, np.float32)
    memory_kv = np.asarray(memory_kv, np.float32)

    qs = q[0] * np.float32(SCALE)       # (H, N, D)
    ks, vs = k[0], v[0]
    ab = attn_bias[0]                   # (NW, W, 2W)
    mvec = mask[0].astype(bool)         # (N,)

    in_maps = [_prep_core(c, qs, ks, vs, ab, mvec) for c in range(NCORES)]
    res = _run_device(in_maps, trace=_trace)
    outs = [r["o"] for r in res.results]            # each (H, 33, WPC*W)

    big = np.stack(outs)                             # (8, H, 33, 2048)
    arr = big.reshape(NCORES, H, 33, WPC, W).transpose(1, 0, 3, 4, 2)
    arr = arr.reshape(H, N, 33)
    num = arr[..., :D].astype(np.float64)            # (H, N, D) window-part numerator
    z = arr[..., D].astype(np.float64)               # (H, N) window-part denominator

    # memory-slot attention (4 keys, no bias, mask=True) on host
    mk, mv = memory_kv[0], memory_kv[1]              # (H, 4, D)
    sim_m = np.einsum('hnd,hmd->hnm', qs, mk, dtype=np.float64)
    pm = np.exp(SOFTCLAMP * np.tanh(sim_m / SOFTCLAMP))
    num = num + np.einsum('hnm,hmd->hnd', pm, mv.astype(np.float64))
    z = z + pm.sum(-1)

    out = (num / z[..., None]).astype(np.float32)[None]   # (1, H, N, D)
    if _ret_res:
        return out, res
    return out
